# revision 9
# baseline (speedup 1.0000x reference)
"""Trainium2 Bass kernel for nn_Encoder (gnn_message_passing).

Per batch sample b (128 total): nodes = x[b] rearranged to [H=128, C*W=2048];
cosine-kNN (K=8) graph per sample; 4 layers of
  h   = relu(((A+I)/9) @ (x @ gw + gb))
  y   = blockdiag_conv(h) + cb ;  BN over global batch stats
  out = BN(y) + x @ rw + rb
Output [128, 16384] = h3.reshape.

Sharding: data-parallel over batch — 16 samples per core on 8 cores; BatchNorm
statistics are all-reduced across cores (one tiny AllReduce per layer).

Numerics: kNN similarity in full fp32 on the PE (exact top-8 via max8 +
threshold); large matmuls in float32r (fp32 storage with mantissa truncated to
13 explicit bits + exact fp32 PSUM accumulation — measured on HW; weights are
pre-truncated on the host so they DMA straight into f32r tiles).

Dataflow is feat-major ("transposed") throughout:  X^T tiles [feat, node] feed
H = X@gw with X^T slices as the stationary operand (node-major H in PSUM);
aggregation  agg^T = H^T (A^T+I)  uses H as stationary and returns feat-major;
relu(x/9 + gb') is fused into the PSUM eviction; the grouped 1x1 conv is 8
block-diagonal 128x128 matmuls; residual matmuls run feat-major as well and
BN-apply + residual-add are fused into their eviction.  Each layer's output is
written to HBM and re-loaded as the next layer's X^T (keeps SBUF pools LIFO).
"""
import os
import numpy as np

import concourse.bacc as bacc
import concourse.mybir as mybir
from concourse.tile import TileContext
from concourse import bass_utils

dt = mybir.dt
F32 = dt.float32
F32R = dt.float32r
AOP = mybir.AluOpType
ACTF = mybir.ActivationFunctionType

BS, C, H, W = 128, 32, 128, 64
D = C * W                  # 2048
NUM_HEADS = 4
EPS = 1e-5
N_CORES = 8
S = BS // N_CORES          # samples per core = 16
NODES = S * H              # nodes per core = 2048
N_TOTAL = BS * H           # 16384 (BatchNorm divisor)
NCH = NODES // 512         # 4 node chunks of 512

LAYER_DIMS = [(D >> i, D >> (i + 1)) for i in range(4)]  # (fin, fout)
NEG_BIG = -1.0e30


def _trunc_f32r(a):
    """fp32 -> f32r-representable (truncate low 10 mantissa bits; matches the
    HW cast measured via a DVE copy into a float32r tile)."""
    a = np.ascontiguousarray(a, dtype=np.float32)
    return (a.view(np.uint32) & np.uint32(0xFFFFFC00)).view(np.float32)


def build_host_inputs(x, params):
    x = np.ascontiguousarray(np.asarray(x, dtype=np.float32))
    assert x.shape == (BS, C, H, W)
    shared = {}
    eye = np.eye(128, dtype=np.float32)
    shared["eye_neg"] = (NEG_BIG * eye).astype(np.float32)
    shared["eye_pos"] = eye.copy()
    e0 = np.zeros((128, 128), dtype=np.float32)
    e0[0, :] = 1.0
    shared["e0"] = _trunc_f32r(e0)

    for li, p in enumerate(params):
        fin, fout = LAYER_DIMS[li]
        fm = fout // 128
        gb = np.asarray(p["gb"], np.float32)
        cw = np.asarray(p["cw"], np.float32)       # [G, NH, NH]
        shared[f"gw{li}"] = _trunc_f32r(np.asarray(p["gw"], np.float32))
        shared[f"rw{li}"] = _trunc_f32r(np.asarray(p["rw"], np.float32))
        gbt = np.zeros((128, fout), dtype=np.float32)
        gbt[0, :] = gb
        shared[f"gbt{li}"] = _trunc_f32r(gbt)

        G = fout // NUM_HEADS
        wb = np.zeros((fm, 128, 128), dtype=np.float32)
        for g in range(G):
            m, off = divmod(g * NUM_HEADS, 128)
            # y[n, g, o] = sum_i h[n, g, i] * cw[g, o, i]
            wb[m, off:off + NUM_HEADS, off:off + NUM_HEADS] = cw[g].T
        shared[f"wb{li}"] = _trunc_f32r(wb)

        cb = np.asarray(p["cb"], np.float32)
        bng = np.asarray(p["bng"], np.float32)
        bnbrb = np.asarray(p["bnb"], np.float32) + np.asarray(p["rb"], np.float32)
        shared[f"cbt{li}"] = np.ascontiguousarray(cb.reshape(fm, 128).T)
        shared[f"bngt{li}"] = np.ascontiguousarray(bng.reshape(fm, 128).T)
        shared[f"bnbrbt{li}"] = np.ascontiguousarray(bnbrb.reshape(fm, 128).T)

    return [dict(shared, x=x[c * S:(c + 1) * S]) for c in range(N_CORES)]


def build_kernel():
    nc = bacc.Bacc("TRN2", target_bir_lowering=False, num_devices=N_CORES)

    x_d = nc.dram_tensor("x", [S, C, H, W], F32, kind="ExternalInput")
    eye_neg_d = nc.dram_tensor("eye_neg", [128, 128], F32, kind="ExternalInput")
    eye_pos_d = nc.dram_tensor("eye_pos", [128, 128], F32, kind="ExternalInput")
    e0_d = nc.dram_tensor("e0", [128, 128], F32R, kind="ExternalInput")
    wd = {}
    for li, (fin, fout) in enumerate(LAYER_DIMS):
        fm = fout // 128
        wd[f"gw{li}"] = nc.dram_tensor(f"gw{li}", [fin, fout], F32R, kind="ExternalInput")
        wd[f"rw{li}"] = nc.dram_tensor(f"rw{li}", [fin, fout], F32R, kind="ExternalInput")
        wd[f"gbt{li}"] = nc.dram_tensor(f"gbt{li}", [128, fout], F32R, kind="ExternalInput")
        wd[f"wb{li}"] = nc.dram_tensor(f"wb{li}", [fm, 128, 128], F32R, kind="ExternalInput")
        for nm in ("cbt", "bngt", "bnbrbt"):
            wd[f"{nm}{li}"] = nc.dram_tensor(f"{nm}{li}", [128, fm], F32, kind="ExternalInput")
    out_d = nc.dram_tensor("out", [NODES, 128], F32, kind="ExternalOutput")

    # per-layer HBM bounce of y (layer 0 only) and layer outputs (all layers)
    y0_d = nc.dram_tensor("y0_bounce", [LAYER_DIMS[0][1], NODES], F32)
    r0_d = nc.dram_tensor("r0_bounce", [LAYER_DIMS[0][1], NODES], F32)
    ol_d = [nc.dram_tensor(f"out{li}_b", [LAYER_DIMS[li][1], NODES], F32R)
            for li in range(3)]
    cc_in = [nc.dram_tensor(f"cc_in{li}", [128, 2 * (fo // 128)], F32)
             for li, (_, fo) in enumerate(LAYER_DIMS)]
    cc_out = [nc.dram_tensor(f"cc_out{li}", [128, 2 * (fo // 128)], F32,
                             addr_space="Shared")
              for li, (_, fo) in enumerate(LAYER_DIMS)]

    with TileContext(nc) as tc:
        _emit(nc, tc, x_d, eye_neg_d, eye_pos_d, e0_d, wd, out_d,
              y0_d, r0_d, ol_d, cc_in, cc_out)
    nc.compile()
    return nc


def _bn_reduce_and_consts(nc, pool, li, fm, s1t, s2t, wd, cc_in, cc_out, rg):
    """Finalize per-core sums, all-reduce, produce per-m-tile BN scale/shift.

    scale = bng * rsqrt(var + eps);  shift = (bnb + rb) - (mean_raw + cb)*scale
    where mean_raw excludes the conv bias cb (constant shift does not change
    the variance and is folded here instead of at conv eviction).
    """
    cbt = pool.tile([128, fm], F32, tag=f"cbt{li}", name=f"cbt{li}")
    bngt = pool.tile([128, fm], F32, tag=f"bngt{li}", name=f"bngt{li}")
    bnbrbt = pool.tile([128, fm], F32, tag=f"bnbrbt{li}", name=f"bnbrbt{li}")
    nc.sync.dma_start(cbt[:], wd[f"cbt{li}"][:])
    nc.sync.dma_start(bngt[:], wd[f"bngt{li}"][:])
    nc.sync.dma_start(bnbrbt[:], wd[f"bnbrbt{li}"][:])

    stats = pool.tile([128, 2 * fm], F32, tag=f"stats{li}", name=f"stats{li}")
    sred = pool.tile([128, 2 * fm], F32, tag=f"sred{li}", name=f"sred{li}")
    for m in range(fm):
        nc.vector.tensor_reduce(stats[:, m:m + 1], s1t[:, m * NCH:(m + 1) * NCH],
                                mybir.AxisListType.X, AOP.add)
        nc.vector.tensor_reduce(stats[:, fm + m:fm + m + 1],
                                s2t[:, m * NCH:(m + 1) * NCH],
                                mybir.AxisListType.X, AOP.add)
    nc.gpsimd.dma_start(cc_in[li][:], stats[:])
    if os.environ.get("KB_NO_CC"):
        nc.gpsimd.dma_start(sred[:], cc_in[li][:])
    else:
        nc.gpsimd.collective_compute(
            "AllReduce", AOP.add, replica_groups=rg,
            ins=[cc_in[li][:].opt()], outs=[cc_out[li][:].opt()])
        nc.gpsimd.dma_start(sred[:], cc_out[li][:])

    scale_t = pool.tile([128, fm], F32, tag=f"scale{li}", name=f"scale{li}")
    shift_t = pool.tile([128, fm], F32, tag=f"shift{li}", name=f"shift{li}")
    tmpp = pool.tile([128, fm], F32, tag=f"tmpp{li}", name=f"tmpp{li}")
    mean_t = pool.tile([128, fm], F32, tag=f"mean{li}", name=f"mean{li}")
    var_t = pool.tile([128, fm], F32, tag=f"var{li}", name=f"var{li}")
    bn_div = float(NODES if os.environ.get("KB_NO_CC") else N_TOTAL)
    nc.vector.tensor_scalar(mean_t[:], sred[:, 0:fm], 1.0 / bn_div, None, op0=AOP.mult)
    nc.vector.tensor_scalar(var_t[:], sred[:, fm:2 * fm], 1.0 / bn_div, None, op0=AOP.mult)
    nc.vector.tensor_tensor(out=tmpp[:], in0=mean_t[:], in1=mean_t[:], op=AOP.mult)
    nc.vector.tensor_tensor(out=var_t[:], in0=var_t[:], in1=tmpp[:], op=AOP.subtract)
    nc.vector.tensor_scalar(var_t[:], var_t[:], EPS, None, op0=AOP.add)
    nc.scalar.sqrt(tmpp[:], var_t[:])
    nc.vector.reciprocal(tmpp[:], tmpp[:])
    nc.vector.tensor_tensor(out=scale_t[:], in0=bngt[:], in1=tmpp[:], op=AOP.mult)
    nc.vector.tensor_tensor(out=tmpp[:], in0=mean_t[:], in1=cbt[:], op=AOP.add)
    nc.vector.tensor_tensor(out=tmpp[:], in0=tmpp[:], in1=scale_t[:], op=AOP.mult)
    nc.vector.tensor_tensor(out=shift_t[:], in0=bnbrbt[:], in1=tmpp[:], op=AOP.subtract)
    return scale_t, shift_t


def _phase0_half(nc, tc, x_d, XTk, MT, half, ident, eye_neg, eye_pos):
    """kNN graph + feat-major f32r transpose for samples [8*half, 8*half+8)."""
    kt0 = D // 128  # 16
    with (
        tc.tile_pool(name=f"p0sb{half}", bufs=2) as sb0,
        tc.tile_pool(name=f"p0sc{half}", bufs=2) as sc0,
        tc.tile_pool(name=f"p0sm{half}", bufs=2) as sm0,
        tc.tile_pool(name=f"p0pt{half}", bufs=2, space="PSUM") as ps_t,
        tc.tile_pool(name=f"p0ps{half}", bufs=2, space="PSUM") as ps_sim,
    ):
        for sl in range(8):
            s = 8 * half + sl
            xs = sb0.tile([128, D], F32, tag="xs", name="xs")
            nc.sync.dma_start(xs[:].rearrange("p (c w) -> p c w", c=C),
                              x_d[s].rearrange("c h w -> h c w"))

            nsq_scr = sc0.tile([128, D], F32, tag="nsq_scr", name="nsq_scr")
            nsq = sm0.tile([128, 1], F32, tag="nsq", name="nsq")
            nc.scalar.activation(nsq_scr[:], xs[:], ACTF.Square,
                                 bias=0.0, scale=1.0, accum_out=nsq[:])
            nrm = sm0.tile([128, 1], F32, tag="nrm", name="nrm")
            inv = sm0.tile([128, 1], F32, tag="inv", name="inv")
            nc.scalar.sqrt(nrm[:], nsq[:])
            nc.vector.reciprocal(inv[:], nrm[:])

            xts = sc0.tile([128, D], F32, tag="xts", name="xts")  # [feat, node]
            for b in range(4):
                pt = ps_t.tile([128, 512], F32, tag="pt", name="pt")
                for j in range(4):
                    kk = 4 * b + j
                    nc.tensor.transpose(
                        pt[:, j * 128:(j + 1) * 128],
                        xs[:, kk * 128:(kk + 1) * 128], ident[:])
                nc.vector.tensor_copy(xts[:, b * 512:(b + 1) * 512], pt[:])
                nc.scalar.copy(
                    XTk[:, 4 * b:4 * b + 4, sl * 128:(sl + 1) * 128],
                    pt[:].rearrange("p (j n) -> p j n", j=4))

            ps = ps_sim.tile([128, 128], F32, tag="psim", name="psim")
            for kk in range(kt0):
                nc.tensor.matmul(ps[:], xts[:, kk * 128:(kk + 1) * 128],
                                 xts[:, kk * 128:(kk + 1) * 128],
                                 start=(kk == 0), stop=(kk == kt0 - 1))
            # row-scale by 1/|x_i|; symmetric sim => transpose gives the
            # column scaling that top-8 ranking actually needs
            S_sb = sm0.tile([128, 128], F32, tag="S_sb", name="S_sb")
            nc.scalar.activation(S_sb[:], ps[:], ACTF.Copy, bias=0.0, scale=inv[:])
            pst = ps_sim.tile([128, 128], F32, tag="pst", name="pst")
            nc.tensor.transpose(pst[:], S_sb[:], ident[:])
            ST = sm0.tile([128, 128], F32, tag="ST", name="ST")
            nc.vector.tensor_tensor(out=ST[:], in0=pst[:], in1=eye_neg[:], op=AOP.add)
            m8 = sm0.tile([128, 8], F32, tag="m8", name="m8")
            nc.vector.max(out=m8[:], in_=ST[:])
            A_sb = sm0.tile([128, 128], F32, tag="A_sb", name="A_sb")
            nc.vector.tensor_scalar(A_sb[:], ST[:], m8[:, 7:8], None, op0=AOP.is_ge)
            pat = ps_sim.tile([128, 128], F32, tag="pat", name="pat")
            nc.tensor.transpose(pat[:], A_sb[:], ident[:])
            nc.vector.tensor_tensor(out=MT[s][:], in0=pat[:], in1=eye_pos[:],
                                    op=AOP.add)


def _dummy_out(nc, out_d, tile):
    nc.sync.dma_start(out_d[0:128, :], tile[:])


def _emit(nc, tc, x_d, eye_neg_d, eye_pos_d, e0_d, wd, out_d,
          y0_d, r0_d, ol_d, cc_in, cc_out):
    rg = [list(range(N_CORES))]
    STOP = os.environ.get("KB_STOP", "")

    lp = tc.alloc_tile_pool(name="longlived", bufs=1)
    eye_neg = lp.tile([128, 128], F32, tag="eyen", name="eyen")
    eye_pos = lp.tile([128, 128], F32, tag="eyep", name="eyep")
    e0 = lp.tile([128, 128], F32R, tag="e0", name="e0")
    nc.sync.dma_start(eye_neg[:], eye_neg_d[:])
    nc.sync.dma_start(eye_pos[:], eye_pos_d[:])
    nc.sync.dma_start(e0[:], e0_d[:])
    ident = eye_pos
    MT = [lp.tile([128, 128], F32R, tag=f"mt{s}", name=f"mt{s}") for s in range(S)]

    # ===================== LAYER 0 (two halves of 8 samples) ==============
    fin, fout = LAYER_DIMS[0]
    kt, fm = fin // 128, fout // 128           # 16, 8
    l0p = tc.alloc_tile_pool(name="l0long", bufs=1)
    gbt0 = l0p.tile([128, fout], F32R, tag="gbt0", name="gbt0")
    wb0 = l0p.tile([128, fm * 128], F32R, tag="wb0", name="wb0")
    nc.sync.dma_start(gbt0[:], wd["gbt0"][:])
    nc.sync.dma_start(wb0[:].rearrange("p (m q) -> p m q", m=fm),
                      wd["wb0"].rearrange("m p q -> p m q"))
    s1t0 = l0p.tile([128, fm * NCH], F32, tag="s1t0", name="s1t0")
    s2t0 = l0p.tile([128, fm * NCH], F32, tag="s2t0", name="s2t0")

    for half in range(2):
        xtp = tc.alloc_tile_pool(name=f"xt0h{half}", bufs=1)
        XT0h = xtp.tile([128, kt * (NODES // 2)], F32R, tag="xt0h", name="xt0h")
        XTk = XT0h[:].rearrange("p (k n) -> p k n", k=kt)
        _phase0_half(nc, tc, x_d, XTk, MT, half, ident, eye_neg, eye_pos)

        if STOP == "p0":
            xtp.release()
            continue
        with (
            tc.tile_pool(name=f"gw0p{half}", bufs=1) as gwp,
            tc.tile_pool(name=f"h0p{half}", bufs=1) as hp,
            tc.tile_pool(name=f"r0p{half}", bufs=2) as rp,
            tc.tile_pool(name=f"sc0p{half}", bufs=2) as scrp,
            tc.tile_pool(name=f"psh0{half}", bufs=1, space="PSUM") as psh,
            tc.tile_pool(name=f"psa0{half}", bufs=2, space="PSUM") as psa,
            tc.tile_pool(name=f"psy0{half}", bufs=2, space="PSUM") as psy,
        ):
            H_t = {sl: hp.tile([128, fout], F32R, tag=f"h{sl}", name=f"h{sl}")
                   for sl in range(8)}
            for ch in range(2):                  # fout chunks of 512
                gw_ch = gwp.tile([128, kt * 512], F32R, tag="gwch", name="gwch")
                nc.sync.dma_start(
                    gw_ch[:].rearrange("p (k f) -> p k f", k=kt),
                    wd["gw0"][:, ch * 512:(ch + 1) * 512]
                    .rearrange("(k p) f -> p k f", p=128))
                for sl in range(8):
                    ph = psh.tile([128, 512], F32, tag=f"ph{sl % 4}",
                                  name=f"ph{sl % 4}")
                    for kk in range(kt):
                        nc.tensor.matmul(
                            ph[:], XTk[:, kk, sl * 128:(sl + 1) * 128],
                            gw_ch[:, kk * 512:(kk + 1) * 512],
                            start=(kk == 0), stop=False)
                    nc.tensor.matmul(ph[:], e0[:],
                                     gbt0[:, ch * 512:(ch + 1) * 512],
                                     start=False, stop=True)
                    nc.scalar.copy(H_t[sl][:, ch * 512:(ch + 1) * 512], ph[:])

            for g in range(2):                   # 4-sample groups in the half
                gs = [4 * g + i for i in range(4)]   # local sample idx
                relu_t = {m: rp.tile([128, 512], F32R, tag=f"r{m}", name=f"r{m}")
                          for m in range(fm)}
                for sl in gs:
                    s = 8 * half + sl
                    for mb in range(fm // 4):
                        pa = psa.tile([128, 512], F32, tag="pa", name="pa")
                        for mo in range(4):
                            m = 4 * mb + mo
                            nc.tensor.matmul(
                                pa[:, mo * 128:(mo + 1) * 128],
                                H_t[sl][:, m * 128:(m + 1) * 128], MT[s][:],
                                start=True, stop=True)
                        for mo in range(4):
                            m = 4 * mb + mo
                            nc.scalar.activation(
                                relu_t[m][:, (sl % 4) * 128:(sl % 4 + 1) * 128],
                                pa[:, mo * 128:(mo + 1) * 128],
                                ACTF.Relu, bias=0.0, scale=1.0 / 9.0)

                nchk = 2 * half + g              # global 512-node chunk id
                for m in range(fm):
                    py = psy.tile([128, 512], F32, tag="py", name="py")
                    nc.tensor.matmul(py[:], wb0[:, m * 128:(m + 1) * 128],
                                     relu_t[m][:], start=True, stop=True)
                    ysc = scrp.tile([128, 512], F32, tag="ysc", name="ysc")
                    nc.scalar.activation(
                        ysc[:], py[:], ACTF.Copy, bias=0.0, scale=1.0,
                        accum_out=s1t0[:, m * NCH + nchk:m * NCH + nchk + 1])
                    sq = scrp.tile([128, 512], F32, tag="sq", name="sq")
                    nc.scalar.activation(
                        sq[:], ysc[:], ACTF.Square, bias=0.0, scale=1.0,
                        accum_out=s2t0[:, m * NCH + nchk:m * NCH + nchk + 1])
                    nc.sync.dma_start(
                        y0_d[m * 128:(m + 1) * 128,
                             nchk * 512:(nchk + 1) * 512], ysc[:])

        # residual matmuls for this half (pre-BN; r bounced to HBM)
        if STOP == "l0abc":
            xtp.release()
            continue
        with (
            tc.tile_pool(name=f"rw0p{half}", bufs=2) as rwp,
            tc.tile_pool(name=f"er0p{half}", bufs=3) as erp,
            tc.tile_pool(name=f"psr0{half}", bufs=2, space="PSUM") as psr,
        ):
            for m in range(fm):
                rw_m = rwp.tile([128, kt * 128], F32R, tag="rwm", name="rwm")
                nc.sync.dma_start(
                    rw_m[:].rearrange("p (k q) -> p k q", k=kt),
                    wd["rw0"][:, m * 128:(m + 1) * 128]
                    .rearrange("(k p) q -> p k q", p=128))
                for gc in range(2):
                    nchk = 2 * half + gc
                    prr = psr.tile([128, 512], F32, tag="prr", name="prr")
                    for kk in range(kt):
                        nc.tensor.matmul(
                            prr[:], rw_m[:, kk * 128:(kk + 1) * 128],
                            XTk[:, kk, gc * 512:(gc + 1) * 512],
                            start=(kk == 0), stop=(kk == kt - 1))
                    rch = erp.tile([128, 512], F32, tag="rch", name="rch")
                    nc.vector.tensor_copy(rch[:], prr[:])
                    nc.sync.dma_start(
                        r0_d[m * 128:(m + 1) * 128,
                             nchk * 512:(nchk + 1) * 512], rch[:])
        xtp.release()

    if STOP in ("p0", "l0abc"):
        _dummy_out(nc, out_d, eye_pos)
        l0p.release()
        lp.release()
        return
    scale0, shift0 = _bn_reduce_and_consts(nc, l0p, 0, fm, s1t0, s2t0,
                                           wd, cc_in, cc_out, rg)
    with tc.tile_pool(name="f0p", bufs=3) as fp:
        for m in range(fm):
            for nchk in range(NCH):
                ych = fp.tile([128, 512], F32, tag="ych", name="ych")
                rch = fp.tile([128, 512], F32, tag="rch2", name="rch2")
                nc.sync.dma_start(ych[:], y0_d[m * 128:(m + 1) * 128,
                                               nchk * 512:(nchk + 1) * 512])
                nc.sync.dma_start(rch[:], r0_d[m * 128:(m + 1) * 128,
                                               nchk * 512:(nchk + 1) * 512])
                tmp = fp.tile([128, 512], F32, tag="tmp", name="tmp")
                nc.vector.tensor_scalar(
                    tmp[:], ych[:], scale0[:, m:m + 1], shift0[:, m:m + 1],
                    op0=AOP.mult, op1=AOP.add)
                och = fp.tile([128, 512], F32R, tag="och", name="och")
                nc.vector.tensor_tensor(out=och[:], in0=tmp[:], in1=rch[:],
                                        op=AOP.add)
                nc.sync.dma_start(
                    ol_d[0][m * 128:(m + 1) * 128,
                            nchk * 512:(nchk + 1) * 512], och[:])
    l0p.release()
    if STOP == "l0":
        _dummy_out(nc, out_d, eye_pos)
        lp.release()
        return

    # ===================== LAYERS 1..3 ====================================
    out3 = None
    outp = tc.alloc_tile_pool(name="out3p", bufs=1)
    out3 = outp.tile([128, NODES], F32, tag="out3", name="out3")

    for li in range(1, 4):
        fin, fout = LAYER_DIMS[li]
        kt, fm = fin // 128, fout // 128
        hcw = min(fout, 512)
        last = (li == 3)

        lay = tc.alloc_tile_pool(name=f"lay{li}", bufs=1)
        XT = lay.tile([128, kt * NODES], F32R, tag=f"xt{li}", name=f"xt{li}")
        for kk in range(kt):
            nc.sync.dma_start(XT[:, kk * NODES:(kk + 1) * NODES],
                              ol_d[li - 1][kk * 128:(kk + 1) * 128, :])
        XTk = XT[:].rearrange("p (k n) -> p k n", k=kt)

        gbt = lay.tile([128, fout], F32R, tag=f"gbt{li}", name=f"gbt{li}")
        nc.sync.dma_start(gbt[:], wd[f"gbt{li}"][:])
        wb = lay.tile([128, fm * 128], F32R, tag=f"wb{li}", name=f"wb{li}")
        nc.sync.dma_start(wb[:].rearrange("p (m q) -> p m q", m=fm),
                          wd[f"wb{li}"].rearrange("m p q -> p m q"))
        s1t = lay.tile([128, fm * NCH], F32, tag=f"s1t{li}", name=f"s1t{li}")
        s2t = lay.tile([128, fm * NCH], F32, tag=f"s2t{li}", name=f"s2t{li}")
        y_sb = lay.tile([128, fm * NODES], F32, tag=f"y{li}", name=f"y{li}")

        with (
            tc.tile_pool(name=f"gwp{li}", bufs=1) as gwp,
            tc.tile_pool(name=f"rel{li}", bufs=1) as relp,
            tc.tile_pool(name=f"hp{li}", bufs=2) as hp,
            tc.tile_pool(name=f"scr{li}", bufs=2) as scrp,
            tc.tile_pool(name=f"psh{li}", bufs=2, space="PSUM") as psh,
            tc.tile_pool(name=f"psa{li}", bufs=2, space="PSUM") as psa,
            tc.tile_pool(name=f"psy{li}", bufs=2, space="PSUM") as psy,
        ):
            gw_res = gwp.tile([128, kt * fout], F32R, tag=f"gw{li}", name=f"gw{li}")
            nc.sync.dma_start(
                gw_res[:].rearrange("p (k f) -> p k f", k=kt),
                wd[f"gw{li}"].rearrange("(k p) f -> p k f", p=128))
            relu_full = relp.tile([128, fm * NODES], F32R, tag=f"relu{li}",
                                  name=f"relu{li}")

            for s in range(S):
                H_t = hp.tile([128, fout], F32R, tag=f"h{s % 2}", name=f"h{s % 2}")
                for ch in range(fout // hcw):
                    ph = psh.tile([128, hcw], F32, tag="ph", name="ph")
                    for kk in range(kt):
                        nc.tensor.matmul(
                            ph[:], XTk[:, kk, s * 128:(s + 1) * 128],
                            gw_res[:, kk * fout + ch * hcw:kk * fout + (ch + 1) * hcw],
                            start=(kk == 0), stop=False)
                    nc.tensor.matmul(ph[:], e0[:], gbt[:, ch * hcw:(ch + 1) * hcw],
                                     start=False, stop=True)
                    nc.scalar.copy(H_t[:, ch * hcw:(ch + 1) * hcw], ph[:])

                nmb = max(1, fm // 4)
                for mb in range(nmb):
                    mlo = 4 * mb
                    mhi = min(fm, mlo + 4)
                    pa = psa.tile([128, (mhi - mlo) * 128], F32, tag="pa", name="pa")
                    for m in range(mlo, mhi):
                        nc.tensor.matmul(
                            pa[:, (m - mlo) * 128:(m - mlo + 1) * 128],
                            H_t[:, m * 128:(m + 1) * 128], MT[s][:],
                            start=True, stop=True)
                    for m in range(mlo, mhi):
                        nc.scalar.activation(
                            relu_full[:, m * NODES + s * 128:m * NODES + (s + 1) * 128],
                            pa[:, (m - mlo) * 128:(m - mlo + 1) * 128],
                            ACTF.Relu, bias=0.0, scale=1.0 / 9.0)

            for m in range(fm):
                for gc in range(NCH):
                    py = psy.tile([128, 512], F32, tag="py", name="py")
                    nc.tensor.matmul(
                        py[:], wb[:, m * 128:(m + 1) * 128],
                        relu_full[:, m * NODES + gc * 512:m * NODES + (gc + 1) * 512],
                        start=True, stop=True)
                    nc.scalar.activation(
                        y_sb[:, m * NODES + gc * 512:m * NODES + (gc + 1) * 512],
                        py[:], ACTF.Copy, bias=0.0, scale=1.0,
                        accum_out=s1t[:, m * NCH + gc:m * NCH + gc + 1])
                    sq = scrp.tile([128, 512], F32, tag="sq", name="sq")
                    nc.scalar.activation(
                        sq[:], y_sb[:, m * NODES + gc * 512:m * NODES + (gc + 1) * 512],
                        ACTF.Square, bias=0.0, scale=1.0,
                        accum_out=s2t[:, m * NCH + gc:m * NCH + gc + 1])

        scale_t, shift_t = _bn_reduce_and_consts(nc, lay, li, fm, s1t, s2t,
                                                 wd, cc_in, cc_out, rg)

        with (
            tc.tile_pool(name=f"rwp{li}", bufs=1) as rwp,
            tc.tile_pool(name=f"ep{li}", bufs=3) as ep,
            tc.tile_pool(name=f"psr{li}", bufs=2, space="PSUM") as psr,
        ):
            rw_res = rwp.tile([128, kt * fout], F32R, tag=f"rw{li}", name=f"rw{li}")
            nc.sync.dma_start(
                rw_res[:].rearrange("p (k f) -> p k f", k=kt),
                wd[f"rw{li}"].rearrange("(k p) f -> p k f", p=128))
            for m in range(fm):
                for nchk in range(NCH):
                    prr = psr.tile([128, 512], F32, tag="prr", name="prr")
                    for kk in range(kt):
                        nc.tensor.matmul(
                            prr[:],
                            rw_res[:, kk * fout + m * 128:kk * fout + (m + 1) * 128],
                            XTk[:, kk, nchk * 512:(nchk + 1) * 512],
                            start=(kk == 0), stop=(kk == kt - 1))
                    tmp = ep.tile([128, 512], F32, tag="tmp", name="tmp")
                    nc.vector.tensor_scalar(
                        tmp[:],
                        y_sb[:, m * NODES + nchk * 512:m * NODES + (nchk + 1) * 512],
                        scale_t[:, m:m + 1], shift_t[:, m:m + 1],
                        op0=AOP.mult, op1=AOP.add)
                    if last:
                        nc.vector.tensor_tensor(
                            out=out3[:, nchk * 512:(nchk + 1) * 512],
                            in0=tmp[:], in1=prr[:], op=AOP.add)
                    else:
                        och = ep.tile([128, 512], F32R, tag="och", name="och")
                        nc.vector.tensor_tensor(out=och[:], in0=tmp[:], in1=prr[:],
                                                op=AOP.add)
                        nc.sync.dma_start(
                            ol_d[li][m * 128:(m + 1) * 128,
                                     nchk * 512:(nchk + 1) * 512], och[:])
        lay.release()
        if STOP == f"l{li}":
            _dummy_out(nc, out_d, eye_pos)
            outp.release()
            lp.release()
            return

    # ---------------- final transpose to node-major output ----------------
    with (
        tc.tile_pool(name="fin_sb", bufs=2) as fsb,
        tc.tile_pool(name="fin_ps", bufs=2, space="PSUM") as fps,
    ):
        for s in range(S):
            pf = fps.tile([128, 128], F32, tag="pf", name="pf")
            nc.tensor.transpose(pf[:], out3[:, s * 128:(s + 1) * 128], ident[:])
            ot = fsb.tile([128, 128], F32, tag="ot", name="ot")
            nc.vector.tensor_copy(ot[:], pf[:])
            nc.sync.dma_start(out_d[s * 128:(s + 1) * 128, :], ot[:])
    outp.release()
    lp.release()


_CACHED_NC = None


def kernel(**inputs) -> np.ndarray:
    global _CACHED_NC
    x = np.asarray(inputs["x"])
    params = inputs["params"]
    in_maps = build_host_inputs(x, params)
    if _CACHED_NC is None:
        _CACHED_NC = build_kernel()
    res = bass_utils.run_bass_kernel_spmd(
        _CACHED_NC, in_maps, core_ids=list(range(N_CORES)))
    outs = [res.results[c]["out"].reshape(S, H * 128) for c in range(N_CORES)]
    return np.concatenate(outs, axis=0).astype(np.float32)


if __name__ == "__main__":
    build_kernel()
    print("kernel built ok")
